# revision 15
# baseline (speedup 1.0000x reference)
"""Trainium2 Bass kernel for nn_MultiHeadAttention_55894704390646.

Multi-head causal attention, B=2, S=2048, E=1024, H=16 heads, D=64.
Sharding: data-parallel over batch (2 groups) x tensor-parallel over heads
(4 heads per core). Each core computes a partial output-projection result
(row-split Wo); the host sums the 4 partials per batch and adds the bias.

Final design (per core), ~170us HW exec (baseline 215us):
  - x/Wq/Wk/Wv and attention operands bf16 (PSUM fp32); ctx/Wo f32r; out
    partials fp16.
  - reversed chunk order (J=3..0) -> smallest attention chunk last.
  - causal column restriction + gpsimd affine_select for the diagonal
    triangle (no mask tensors, nothing on DVE).
  - ACT runs the exp stream (the attention pace-setter) plus the second
    den-row stage and late out evacuations; DVE does the other PSUM
    evacuations, reciprocals and normalize multiplies.
  - AVs lag scores by 2 at pair starts so the previous pair's normalize
    chain (den->recip->broadcast->mult) overlaps PE work; projection and
    lagged output-projection groups fill remaining PE slack.
  - inputs stream over both hardware DMA queues (sync + scalar), first
    x chunk split across them; out written one DMA per token block.
"""

import sys

if "/opt/trn_rl_repo" not in sys.path:
    sys.path.insert(0, "/opt/trn_rl_repo")

import numpy as np
import ml_dtypes

import concourse.bass as bass
from concourse import bacc
import concourse.mybir as mybir
import concourse.tile as tile
from concourse.bass_utils import run_bass_kernel_spmd

B, S, E, H, D = 2, 2048, 1024, 16, 64
N_CORES = 8
DP = 2                 # batch groups
TP = 4                 # cores per batch group
HL = H // TP           # local heads per core = 4
DL = HL * D            # local head dims = 256
P = 128
NTB = S // P           # token blocks = 16
QC = 512               # query chunk
NQC = S // QC          # query chunks = 4
NKB = QC // P          # k-blocks per q chunk = 4
NPAIR = HL // 2        # head pairs = 2
NEO = E // QC          # output feature chunks of 512 = 2
NKO = E // P           # contraction blocks over E = 8

f32 = mybir.dt.float32
f32r = mybir.dt.float32r
f16 = mybir.dt.float16
bf16 = mybir.dt.bfloat16
EXP = mybir.ActivationFunctionType.Exp
MULT = mybir.AluOpType.mult

_NC_CACHE = None


def _build_nc():
    nc = bacc.Bacc("TRN2", target_bir_lowering=False, debug=False)

    xT = nc.dram_tensor("xT", (E, S), bf16, kind="ExternalInput")
    wqT = nc.dram_tensor("wqT", (E, DL), bf16, kind="ExternalInput")
    wkT = nc.dram_tensor("wkT", (E, DL), bf16, kind="ExternalInput")
    wvT = nc.dram_tensor("wvT", (E, DL), bf16, kind="ExternalInput")
    woT = nc.dram_tensor("woT", (DL, E), f32r, kind="ExternalInput")
    out = nc.dram_tensor("out", (S, E), f16, kind="ExternalOutput")

    with tile.TileContext(nc) as tc:
        with (
            nc.allow_low_precision(reason="bf16/f32r matmuls, fp16 partials"),
            tc.tile_pool(name="big", bufs=1) as big,
            tc.tile_pool(name="work", bufs=6) as work,
            tc.tile_pool(name="work2", bufs=2) as work2,
            tc.tile_pool(name="osb", bufs=3) as osb,
            tc.tile_pool(name="ps_mm", bufs=2, space="PSUM") as ps_mm,
            tc.tile_pool(name="ps_s", bufs=2, space="PSUM") as ps_s,
            tc.tile_pool(name="ps_ctx", bufs=2, space="PSUM") as ps_ctx,
        ):
            # ---- input DMAs on the two hardware DGE queues (sync+scalar),
            # ordered by first consumer; the first x chunk is split across
            # both queues so kproj(0) can start earliest.
            xT_r = xT[:].rearrange("(ko p) (c s) -> p ko c s", p=P, c=NQC)
            wkT_sb = big.tile([P, NKO, DL], bf16, tag="wkT")
            nc.sync.dma_start(wkT_sb[:], wkT[:].rearrange("(ko p) d -> p ko d", p=P))
            xc = [
                big.tile([P, NKO, QC], bf16, tag=f"xc{c}", name=f"xc{c}")
                for c in range(NQC)
            ]
            nc.scalar.dma_start(xc[0][:, 0:4, :], xT_r[:, 0:4, 0, :])
            nc.sync.dma_start(xc[0][:, 4:8, :], xT_r[:, 4:8, 0, :])
            wqT_sb = big.tile([P, NKO, DL], bf16, tag="wqT")
            nc.scalar.dma_start(wqT_sb[:], wqT[:].rearrange("(ko p) d -> p ko d", p=P))
            nc.sync.dma_start(xc[3][:, 4:8, :], xT_r[:, 4:8, 3, :])
            nc.scalar.dma_start(xc[3][:, 0:4, :], xT_r[:, 0:4, 3, :])
            wvT_sb = big.tile([P, NKO, DL], bf16, tag="wvT")
            nc.scalar.dma_start(wvT_sb[:], wvT[:].rearrange("(ko p) d -> p ko d", p=P))
            nc.sync.dma_start(xc[1][:], xT_r[:, :, 1, :])
            nc.scalar.dma_start(xc[2][:], xT_r[:, :, 2, :])
            woT_sb = big.tile([P, NPAIR, E], f32r, tag="woT")
            nc.sync.dma_start(woT_sb[:], woT[:].rearrange("(pr p) e -> p pr e", p=P))

            # persistent activation buffers
            qT_c = [[None] * NQC for _ in range(NPAIR)]
            kT_c = [[None] * NQC for _ in range(NPAIR)]
            for pr in range(NPAIR):
                for ch in range(NQC):
                    qT_c[pr][ch] = big.tile(
                        [P, QC], bf16, tag=f"qT{pr}{ch}", name=f"qT{pr}{ch}"
                    )
                    kT_c[pr][ch] = big.tile(
                        [P, QC], bf16, tag=f"kT{pr}{ch}", name=f"kT{pr}{ch}"
                    )
            v_tb = []
            for tb in range(NTB):
                vt = big.tile([P, HL, D + 1], bf16, tag=f"v{tb}", name=f"v{tb}")
                nc.gpsimd.memset(vt[:, :, D], 1.0)
                v_tb.append(vt)
            ctx_J = []
            for J in range(NQC):
                ctx_J.append(
                    big.tile([P, NPAIR, QC], f32r, tag=f"ctxT{J}", name=f"ctxT{J}")
                )

            def emit_proj_half(wt_sb, dst, pr, ch):
                """One pair's q/k projection for one chunk: 8 MMs + evac."""
                pp = ps_mm.tile([P, QC], f32, tag="mm", name=f"pp_{pr}_{ch}")
                for ko in range(NKO):
                    nc.tensor.matmul(
                        pp[:],
                        wt_sb[:, ko, pr * P : (pr + 1) * P],
                        xc[ch][:, ko, :],
                        start=(ko == 0),
                        stop=(ko == NKO - 1),
                    )
                nc.vector.tensor_copy(dst[pr][ch][:], pp[:])

            def emit_v(tb):
                pv_full = ps_mm.tile([P, QC], f32, tag="mm", name=f"pv{tb}")
                pv = pv_full[:, 0:DL]
                for ko in range(NKO):
                    nc.tensor.matmul(
                        pv[:],
                        xc[tb // NKB][:, ko, (tb % NKB) * P : (tb % NKB + 1) * P],
                        wvT_sb[:, ko, :],
                        start=(ko == 0),
                        stop=(ko == NKO - 1),
                    )
                nc.vector.tensor_copy(
                    v_tb[tb][:, :, 0:D],
                    pv[:].rearrange("p (h d) -> p h d", h=HL),
                )

            def scores_group(pr, J, I):
                """Scores + exp (+ causal triangle zero) for k-block I.
                Diagonal blocks only compute query columns >= 128*di."""
                di = I - NKB * J
                q0 = P * di if di >= 0 else 0
                kch = I // NKB
                ik = slice((I % NKB) * P, (I % NKB + 1) * P)
                s = ps_s.tile([P, 2, QC], f32, tag="s", name="s")
                nc.tensor.matmul(
                    s[:, 0, q0:QC],
                    kT_c[pr][kch][0:64, ik],
                    qT_c[pr][J][0:64, q0:QC],
                    start=True,
                    stop=True,
                )
                nc.tensor.matmul(
                    s[:, 1, q0:QC],
                    kT_c[pr][kch][64:128, ik],
                    qT_c[pr][J][64:128, q0:QC],
                    start=True,
                    stop=True,
                )
                pT = work.tile([P, 2, QC], bf16, tag="pT", name="pT")
                nc.scalar.activation(pT[:, :, q0:QC], s[:, :, q0:QC], EXP, scale=0.125)
                if di >= 0:
                    nc.gpsimd.affine_select(
                        out=pT[:, :, q0 : q0 + P],
                        in_=pT[:, :, q0 : q0 + P],
                        compare_op=mybir.AluOpType.is_ge,
                        fill=0.0,
                        base=0,
                        pattern=[[0, 2], [1, P]],
                        channel_multiplier=-1,
                    )
                return (pT, q0)

            def emit_out_block(J, tb):
                """Output projection for one 128-token block: both feature
                halves accumulated, one evac each, ONE dma."""
                o_sb = osb.tile([P, E], f16, tag="o_sb")
                tsl = slice((tb % NKB) * P, (tb % NKB + 1) * P)
                for ec in range(NEO):
                    o_ps = ps_mm.tile([P, QC], f32, tag="mm", name=f"o{J}_{tb}_{ec}")
                    for pr in range(NPAIR):
                        nc.tensor.matmul(
                            o_ps[:],
                            ctx_J[J][:, pr, tsl],
                            woT_sb[:, pr, ec * QC : (ec + 1) * QC],
                            start=(pr == 0),
                            stop=(pr == NPAIR - 1),
                        )
                    if J <= 1:
                        nc.scalar.copy(o_sb[:, ec * QC : (ec + 1) * QC], o_ps[:])
                    else:
                        nc.vector.tensor_copy(
                            o_sb[:, ec * QC : (ec + 1) * QC], o_ps[:]
                        )
                nc.sync.dma_start(out[tb * P : (tb + 1) * P, :], o_sb[:])

            filler_q = []

            def enqueue_out(J):
                for tb in range(NKB * J, NKB * (J + 1)):
                    filler_q.append(lambda J=J, tb=tb: emit_out_block(J, tb))

            def emit_attn_pair(pr, J, fillers=None, trailing=None):
                """Attention for head pair (2pr, 2pr+1) on query chunk J.
                AVs lag scores by 2 for the first iterations (boundary
                decoupling from the previous pair's normalize chain), then
                by 1. Fillers keep the PE fed at the ACT exp pace."""
                h0, h1 = 2 * pr, 2 * pr + 1
                nI = NKB * (J + 1)
                ctx = [
                    ps_ctx.tile([D + 1, QC], f32, tag="ctx", name="ctx0"),
                    ps_ctx.tile([D + 1, QC], f32, tag="ctx", name="ctx1"),
                ]

                def emit_av(I, pTq):
                    pT, q0 = pTq
                    nc.tensor.matmul(
                        ctx[0][:, q0:QC], v_tb[I][:, h0, :], pT[:, 0, q0:QC],
                        start=(I == 0), stop=(I == nI - 1),
                        skip_group_check=True,
                    )
                    nc.tensor.matmul(
                        ctx[1][:, q0:QC], v_tb[I][:, h1, :], pT[:, 1, q0:QC],
                        start=(I == 0), stop=(I == nI - 1),
                        skip_group_check=True,
                    )

                def filler(I):
                    if fillers and I in fillers:
                        for th in fillers[I]:
                            th()
                    elif filler_q and I % 2 == 0 and 4 <= I:
                        filler_q.pop(0)()

                pTs = [pending.pop() if pending else scores_group(pr, J, 0)]
                nxt_av = 0
                for I in range(1, nI):
                    pTs.append(scores_group(pr, J, I))
                    filler(I)
                    if I >= 2:
                        emit_av(nxt_av, pTs.pop(0))
                        nxt_av += 1
                nxt = chain.pop(0) if chain else None
                if nxt is not None:
                    pending.append(scores_group(nxt[0], nxt[1], 0))
                while pTs:
                    emit_av(nxt_av, pTs.pop(0))
                    nxt_av += 1
                if trailing:
                    for th in trailing:
                        th()

                # normalize as two independent per-head chains so each ctx
                # bank frees as early as possible. den staged DVE(h0)/ACT(h1)
                # in parallel; per-head recip + broadcast + multiply.
                ctx0, ctx1 = ctx

                def norm_head(r, cx, first):
                    den_sb = work2.tile([1, QC], f32, tag=f"den{r}", name=f"den{r}")
                    eng = nc.vector if r == 0 else nc.scalar
                    (eng.tensor_copy if r == 0 else eng.copy)(
                        den_sb[:], cx[D : D + 1, :]
                    )
                    rec = work2.tile([1, QC], f32, tag=f"rec{r}", name=f"rec{r}")
                    nc.vector.reciprocal_approx_fast(rec[:], den_sb[:])
                    dnb = work2.tile([64, QC], f32, tag=f"dnb{r}", name=f"dnb{r}")
                    nc.gpsimd.partition_broadcast(dnb[:], rec[:])
                    if r == 0:
                        nc.vector.tensor_tensor(
                            ctx_J[J][0:64, pr, :], cx[0:D, :], dnb[:], MULT
                        )
                    else:
                        tmp = work2.tile([64, QC], f32r, tag="ctmp")
                        nc.vector.tensor_tensor(tmp[:], cx[0:D, :], dnb[:], MULT)
                        nc.sync.dma_start(ctx_J[J][64:128, pr, :], tmp[:])

                if J == 0:
                    norm_head(1, ctx1, True)
                    norm_head(0, ctx0, False)
                else:
                    norm_head(0, ctx0, True)
                    norm_head(1, ctx1, False)

            # ---- schedule: reversed chunk order, fillers keep PE dense
            chain = [(0, 3), (1, 3), (0, 2), (1, 2), (0, 1), (1, 1), (0, 0), (1, 0)]
            pending = []

            emit_proj_half(wkT_sb, kT_c, 0, 0)
            emit_proj_half(wkT_sb, kT_c, 1, 0)
            emit_proj_half(wqT_sb, qT_c, 0, 3)
            emit_proj_half(wqT_sb, qT_c, 1, 3)
            for tb in range(4):
                emit_v(tb)
            chain.pop(0)
            emit_attn_pair(0, 3, fillers={
                1: [lambda: emit_v(4), lambda: emit_v(5)],
                2: [lambda: emit_proj_half(wkT_sb, kT_c, 0, 1),
                    lambda: emit_proj_half(wkT_sb, kT_c, 1, 1)],
                3: [lambda: emit_v(6), lambda: emit_v(7)],
                4: [lambda: emit_v(8)],
                5: [lambda: emit_v(9)],
                6: [lambda: emit_proj_half(wkT_sb, kT_c, 0, 2),
                    lambda: emit_proj_half(wkT_sb, kT_c, 1, 2)],
                7: [lambda: emit_v(10), lambda: emit_v(11)],
                8: [lambda: emit_v(12)],
                9: [lambda: emit_v(13)],
                10: [lambda: emit_proj_half(wkT_sb, kT_c, 0, 3),
                     lambda: emit_proj_half(wkT_sb, kT_c, 1, 3)],
                11: [lambda: emit_v(14)],
                12: [lambda: emit_v(15)],
            }, trailing=[lambda: emit_proj_half(wqT_sb, qT_c, 0, 2)])
            emit_attn_pair(1, 3, fillers={
                6: [lambda: emit_proj_half(wqT_sb, qT_c, 1, 2)],
            })
            enqueue_out(3)
            emit_attn_pair(0, 2, fillers={
                1: [lambda: emit_proj_half(wqT_sb, qT_c, 0, 1)],
                6: [lambda: emit_proj_half(wqT_sb, qT_c, 1, 1)],
            })
            emit_attn_pair(1, 2)
            enqueue_out(2)
            emit_attn_pair(0, 1, fillers={
                1: [lambda: emit_proj_half(wqT_sb, qT_c, 0, 0)],
                4: [lambda: emit_proj_half(wqT_sb, qT_c, 1, 0)],
            })
            emit_attn_pair(1, 1)
            enqueue_out(1)
            lazy = lambda: (filler_q.pop(0)() if filler_q else None)
            emit_attn_pair(0, 0, fillers={1: [lazy], 2: [lazy]})
            emit_attn_pair(1, 0, fillers={1: [lazy], 2: [lazy]})
            enqueue_out(0)
            while filler_q:
                filler_q.pop(0)()

    nc.compile()
    return nc


def get_nc():
    global _NC_CACHE
    if _NC_CACHE is None:
        _NC_CACHE = _build_nc()
    return _NC_CACHE


def _round_fp32r(a):
    """Round-to-nearest-even onto the fp32r grid (11 mantissa bits)."""
    b = np.ascontiguousarray(a, dtype=np.float32).view(np.uint32)
    b = b + 0x7FF + ((b >> 12) & 1)
    b &= np.uint32(0xFFFFF000)
    return b.view(np.float32)


def _bf16(a):
    return np.ascontiguousarray(a, dtype=np.float32).astype(ml_dtypes.bfloat16)


def make_in_maps(x, Wq, Wk, Wv, Wo):
    x = np.asarray(x, dtype=np.float32)
    Wq = np.asarray(Wq, dtype=np.float32)
    Wk = np.asarray(Wk, dtype=np.float32)
    Wv = np.asarray(Wv, dtype=np.float32)
    Wo = np.asarray(Wo, dtype=np.float32)
    in_maps = []
    for c in range(N_CORES):
        b, g = divmod(c, TP)
        sl = slice(DL * g, DL * (g + 1))
        in_maps.append(
            {
                "xT": _bf16(x[b].T),
                "wqT": _bf16(Wq[sl].T),
                "wkT": _bf16(Wk[sl].T),
                "wvT": _bf16(Wv[sl].T),
                "woT": _round_fp32r(Wo[:, sl].T),
            }
        )
    return in_maps


def _combine(results, bo):
    bo = np.asarray(bo, dtype=np.float32)
    y = np.zeros((B, S, E), dtype=np.float32)
    for c in range(N_CORES):
        y[c // TP] += results[c]["out"].astype(np.float32)
    y += bo
    return y


def kernel(x, Wq, Wk, Wv, Wo, bo):
    nc = get_nc()
    in_maps = make_in_maps(x, Wq, Wk, Wv, Wo)
    res = run_bass_kernel_spmd(nc, in_maps, list(range(N_CORES)))
    return _combine(res.results, bo)


def kernel_traced(x, Wq, Wk, Wv, Wo, bo, trace_cores=None):
    """Like kernel() but with NTFF tracing; returns (output, BassKernelResults)."""
    nc = get_nc()
    in_maps = make_in_maps(x, Wq, Wk, Wv, Wo)
    res = run_bass_kernel_spmd(
        nc, in_maps, list(range(N_CORES)), trace=True, trace_cores=trace_cores
    )
    return _combine(res.results, bo), res


# revision 16
# speedup vs baseline: 1.0598x; 1.0598x over previous
"""Trainium2 Bass kernel for nn_MultiHeadAttention_55894704390646.

Multi-head causal attention, B=2, S=2048, E=1024, H=16 heads, D=64.
Sharding: data-parallel over batch (2 groups) x tensor-parallel over heads
(4 heads per core). Each core computes a partial output-projection result
(row-split Wo); the host sums the 4 partials per batch and adds the bias.

Final design (per core), ~170us HW exec (baseline 215us):
  - x/Wq/Wk/Wv and attention operands bf16 (PSUM fp32); ctx/Wo f32r; out
    partials fp16.
  - reversed chunk order (J=3..0) -> smallest attention chunk last.
  - causal column restriction + gpsimd affine_select for the diagonal
    triangle (no mask tensors, nothing on DVE).
  - ACT runs the exp stream (the attention pace-setter) plus the second
    den-row stage and late out evacuations; DVE does the other PSUM
    evacuations, reciprocals and normalize multiplies.
  - AVs lag scores by 2 at pair starts so the previous pair's normalize
    chain (den->recip->broadcast->mult) overlaps PE work; projection and
    lagged output-projection groups fill remaining PE slack.
  - inputs stream over both hardware DMA queues (sync + scalar), first
    x chunk split across them; out written one DMA per token block.
"""

import sys

if "/opt/trn_rl_repo" not in sys.path:
    sys.path.insert(0, "/opt/trn_rl_repo")

import numpy as np
import ml_dtypes

import concourse.bass as bass
from concourse import bacc
import concourse.mybir as mybir
import concourse.tile as tile
from concourse.bass_utils import run_bass_kernel_spmd

B, S, E, H, D = 2, 2048, 1024, 16, 64
N_CORES = 8
DP = 2                 # batch groups
TP = 4                 # cores per batch group
HL = H // TP           # local heads per core = 4
DL = HL * D            # local head dims = 256
P = 128
NTB = S // P           # token blocks = 16
QC = 512               # query chunk
NQC = S // QC          # query chunks = 4
NKB = QC // P          # k-blocks per q chunk = 4
NPAIR = HL // 2        # head pairs = 2
NEO = E // QC          # output feature chunks of 512 = 2
NKO = E // P           # contraction blocks over E = 8

f32 = mybir.dt.float32
f32r = mybir.dt.float32r
f16 = mybir.dt.float16
bf16 = mybir.dt.bfloat16
EXP = mybir.ActivationFunctionType.Exp
MULT = mybir.AluOpType.mult

_NC_CACHE = None


def _build_nc():
    nc = bacc.Bacc("TRN2", target_bir_lowering=False, debug=False)

    xT = nc.dram_tensor("xT", (E, S), bf16, kind="ExternalInput")
    wqT = nc.dram_tensor("wqT", (E, DL), bf16, kind="ExternalInput")
    wkT = nc.dram_tensor("wkT", (E, DL), bf16, kind="ExternalInput")
    wvT = nc.dram_tensor("wvT", (E, DL), bf16, kind="ExternalInput")
    woT = nc.dram_tensor("woT", (DL, E), f32r, kind="ExternalInput")
    out = nc.dram_tensor("out", (S, E), f16, kind="ExternalOutput")

    with tile.TileContext(nc) as tc:
        with (
            nc.allow_low_precision(reason="bf16/f32r matmuls, fp16 partials"),
            tc.tile_pool(name="big", bufs=1) as big,
            tc.tile_pool(name="work", bufs=6) as work,
            tc.tile_pool(name="work2", bufs=2) as work2,
            tc.tile_pool(name="osb", bufs=3) as osb,
            tc.tile_pool(name="ps_mm", bufs=2, space="PSUM") as ps_mm,
            tc.tile_pool(name="ps_s", bufs=2, space="PSUM") as ps_s,
            tc.tile_pool(name="ps_ctx", bufs=2, space="PSUM") as ps_ctx,
        ):
            # ---- input DMAs on the two hardware DGE queues (sync+scalar),
            # ordered by first consumer; the first x chunk is split across
            # both queues so kproj(0) can start earliest.
            xT_r = xT[:].rearrange("(ko p) (c s) -> p ko c s", p=P, c=NQC)
            wkT_sb = big.tile([P, NKO, DL], bf16, tag="wkT")
            nc.sync.dma_start(wkT_sb[:], wkT[:].rearrange("(ko p) d -> p ko d", p=P))
            xc = [
                big.tile([P, NKO, QC], bf16, tag=f"xc{c}", name=f"xc{c}")
                for c in range(NQC)
            ]
            nc.scalar.dma_start(xc[0][:, 0:4, :], xT_r[:, 0:4, 0, :])
            nc.sync.dma_start(xc[0][:, 4:8, :], xT_r[:, 4:8, 0, :])
            wqT_sb = big.tile([P, NKO, DL], bf16, tag="wqT")
            nc.scalar.dma_start(wqT_sb[:], wqT[:].rearrange("(ko p) d -> p ko d", p=P))
            nc.sync.dma_start(xc[3][:], xT_r[:, :, 3, :])
            wvT_sb = big.tile([P, NKO, DL], bf16, tag="wvT")
            nc.scalar.dma_start(wvT_sb[:], wvT[:].rearrange("(ko p) d -> p ko d", p=P))
            nc.sync.dma_start(xc[1][:], xT_r[:, :, 1, :])
            nc.scalar.dma_start(xc[2][:], xT_r[:, :, 2, :])
            woT_sb = big.tile([P, NPAIR, E], f32r, tag="woT")
            nc.sync.dma_start(woT_sb[:], woT[:].rearrange("(pr p) e -> p pr e", p=P))

            # persistent activation buffers
            qT_c = [[None] * NQC for _ in range(NPAIR)]
            kT_c = [[None] * NQC for _ in range(NPAIR)]
            for pr in range(NPAIR):
                for ch in range(NQC):
                    qT_c[pr][ch] = big.tile(
                        [P, QC], bf16, tag=f"qT{pr}{ch}", name=f"qT{pr}{ch}"
                    )
                    kT_c[pr][ch] = big.tile(
                        [P, QC], bf16, tag=f"kT{pr}{ch}", name=f"kT{pr}{ch}"
                    )
            v_tb = []
            for tb in range(NTB):
                vt = big.tile([P, HL, D + 1], bf16, tag=f"v{tb}", name=f"v{tb}")
                nc.gpsimd.memset(vt[:, :, D], 1.0)
                v_tb.append(vt)
            ctx_J = []
            for J in range(NQC):
                ctx_J.append(
                    big.tile([P, NPAIR, QC], f32r, tag=f"ctxT{J}", name=f"ctxT{J}")
                )

            def emit_proj_half(wt_sb, dst, pr, ch):
                """One pair's q/k projection for one chunk: 8 MMs + evac."""
                pp = ps_mm.tile([P, QC], f32, tag="mm", name=f"pp_{pr}_{ch}")
                for ko in range(NKO):
                    nc.tensor.matmul(
                        pp[:],
                        wt_sb[:, ko, pr * P : (pr + 1) * P],
                        xc[ch][:, ko, :],
                        start=(ko == 0),
                        stop=(ko == NKO - 1),
                    )
                nc.vector.tensor_copy(dst[pr][ch][:], pp[:])

            def emit_v(tb):
                pv_full = ps_mm.tile([P, QC], f32, tag="mm", name=f"pv{tb}")
                pv = pv_full[:, 0:DL]
                for ko in range(NKO):
                    nc.tensor.matmul(
                        pv[:],
                        xc[tb // NKB][:, ko, (tb % NKB) * P : (tb % NKB + 1) * P],
                        wvT_sb[:, ko, :],
                        start=(ko == 0),
                        stop=(ko == NKO - 1),
                    )
                nc.vector.tensor_copy(
                    v_tb[tb][:, :, 0:D],
                    pv[:].rearrange("p (h d) -> p h d", h=HL),
                )

            def scores_group(pr, J, I):
                """Scores + exp (+ causal triangle zero) for k-block I.
                Diagonal blocks only compute query columns >= 128*di."""
                di = I - NKB * J
                q0 = P * di if di >= 0 else 0
                kch = I // NKB
                ik = slice((I % NKB) * P, (I % NKB + 1) * P)
                s = ps_s.tile([P, 2, QC], f32, tag="s", name="s")
                nc.tensor.matmul(
                    s[:, 0, q0:QC],
                    kT_c[pr][kch][0:64, ik],
                    qT_c[pr][J][0:64, q0:QC],
                    start=True,
                    stop=True,
                )
                nc.tensor.matmul(
                    s[:, 1, q0:QC],
                    kT_c[pr][kch][64:128, ik],
                    qT_c[pr][J][64:128, q0:QC],
                    start=True,
                    stop=True,
                )
                pT = work.tile([P, 2, QC], bf16, tag="pT", name="pT")
                nc.scalar.activation(pT[:, :, q0:QC], s[:, :, q0:QC], EXP, scale=0.125)
                if di >= 0:
                    nc.gpsimd.affine_select(
                        out=pT[:, :, q0 : q0 + P],
                        in_=pT[:, :, q0 : q0 + P],
                        compare_op=mybir.AluOpType.is_ge,
                        fill=0.0,
                        base=0,
                        pattern=[[0, 2], [1, P]],
                        channel_multiplier=-1,
                    )
                return (pT, q0)

            def emit_out_block(J, tb):
                """Output projection for one 128-token block: both feature
                halves accumulated, one evac each, ONE dma."""
                o_sb = osb.tile([P, E], f16, tag="o_sb")
                tsl = slice((tb % NKB) * P, (tb % NKB + 1) * P)
                for ec in range(NEO):
                    o_ps = ps_mm.tile([P, QC], f32, tag="mm", name=f"o{J}_{tb}_{ec}")
                    for pr in range(NPAIR):
                        nc.tensor.matmul(
                            o_ps[:],
                            ctx_J[J][:, pr, tsl],
                            woT_sb[:, pr, ec * QC : (ec + 1) * QC],
                            start=(pr == 0),
                            stop=(pr == NPAIR - 1),
                        )
                    if J <= 1:
                        nc.scalar.copy(o_sb[:, ec * QC : (ec + 1) * QC], o_ps[:])
                    else:
                        nc.vector.tensor_copy(
                            o_sb[:, ec * QC : (ec + 1) * QC], o_ps[:]
                        )
                nc.sync.dma_start(out[tb * P : (tb + 1) * P, :], o_sb[:])

            filler_q = []

            def enqueue_out(J):
                for tb in range(NKB * J, NKB * (J + 1)):
                    filler_q.append(lambda J=J, tb=tb: emit_out_block(J, tb))

            def emit_attn_pair(pr, J, fillers=None, trailing=None):
                """Attention for head pair (2pr, 2pr+1) on query chunk J.
                AVs lag scores by 2 for the first iterations (boundary
                decoupling from the previous pair's normalize chain), then
                by 1. Fillers keep the PE fed at the ACT exp pace."""
                h0, h1 = 2 * pr, 2 * pr + 1
                nI = NKB * (J + 1)
                ctx = [
                    ps_ctx.tile([D + 1, QC], f32, tag="ctx", name="ctx0"),
                    ps_ctx.tile([D + 1, QC], f32, tag="ctx", name="ctx1"),
                ]

                def emit_av(I, pTq):
                    pT, q0 = pTq
                    nc.tensor.matmul(
                        ctx[0][:, q0:QC], v_tb[I][:, h0, :], pT[:, 0, q0:QC],
                        start=(I == 0), stop=(I == nI - 1),
                        skip_group_check=True,
                    )
                    nc.tensor.matmul(
                        ctx[1][:, q0:QC], v_tb[I][:, h1, :], pT[:, 1, q0:QC],
                        start=(I == 0), stop=(I == nI - 1),
                        skip_group_check=True,
                    )

                def filler(I):
                    if fillers and I in fillers:
                        for th in fillers[I]:
                            th()
                    elif filler_q and I % 2 == 0 and 4 <= I:
                        filler_q.pop(0)()

                pTs = [pending.pop() if pending else scores_group(pr, J, 0)]
                nxt_av = 0
                for I in range(1, nI):
                    pTs.append(scores_group(pr, J, I))
                    filler(I)
                    if I >= 2:
                        emit_av(nxt_av, pTs.pop(0))
                        nxt_av += 1
                nxt = chain.pop(0) if chain else None
                if nxt is not None:
                    pending.append(scores_group(nxt[0], nxt[1], 0))
                while pTs:
                    emit_av(nxt_av, pTs.pop(0))
                    nxt_av += 1
                if trailing:
                    for th in trailing:
                        th()

                # normalize as two independent per-head chains so each ctx
                # bank frees as early as possible. den staged DVE(h0)/ACT(h1)
                # in parallel; per-head recip + broadcast + multiply.
                ctx0, ctx1 = ctx

                def norm_head(r, cx, first):
                    den_sb = work2.tile([1, QC], f32, tag=f"den{r}", name=f"den{r}")
                    eng = nc.vector if r == 0 else nc.scalar
                    (eng.tensor_copy if r == 0 else eng.copy)(
                        den_sb[:], cx[D : D + 1, :]
                    )
                    rec = work2.tile([1, QC], f32, tag=f"rec{r}", name=f"rec{r}")
                    nc.vector.reciprocal_approx_fast(rec[:], den_sb[:])
                    dnb = work2.tile([64, QC], f32, tag=f"dnb{r}", name=f"dnb{r}")
                    nc.gpsimd.partition_broadcast(dnb[:], rec[:])
                    if r == 0:
                        nc.vector.tensor_tensor(
                            ctx_J[J][0:64, pr, :], cx[0:D, :], dnb[:], MULT
                        )
                    else:
                        tmp = work2.tile([64, QC], f32r, tag="ctmp")
                        nc.vector.tensor_tensor(tmp[:], cx[0:D, :], dnb[:], MULT)
                        nc.sync.dma_start(ctx_J[J][64:128, pr, :], tmp[:])

                if J == 0:
                    norm_head(1, ctx1, True)
                    norm_head(0, ctx0, False)
                else:
                    norm_head(0, ctx0, True)
                    norm_head(1, ctx1, False)

            # ---- schedule: reversed chunk order, fillers keep PE dense
            chain = [(0, 3), (1, 3), (0, 2), (1, 2), (0, 1), (1, 1), (0, 0), (1, 0)]
            pending = []

            emit_proj_half(wkT_sb, kT_c, 0, 0)
            emit_proj_half(wkT_sb, kT_c, 1, 0)
            emit_proj_half(wqT_sb, qT_c, 0, 3)
            emit_proj_half(wqT_sb, qT_c, 1, 3)
            for tb in range(4):
                emit_v(tb)
            chain.pop(0)
            emit_attn_pair(0, 3, fillers={
                1: [lambda: emit_v(4), lambda: emit_v(5)],
                2: [lambda: emit_proj_half(wkT_sb, kT_c, 0, 1),
                    lambda: emit_proj_half(wkT_sb, kT_c, 1, 1)],
                3: [lambda: emit_v(6), lambda: emit_v(7)],
                4: [lambda: emit_v(8)],
                5: [lambda: emit_v(9)],
                6: [lambda: emit_proj_half(wkT_sb, kT_c, 0, 2),
                    lambda: emit_proj_half(wkT_sb, kT_c, 1, 2)],
                7: [lambda: emit_v(10), lambda: emit_v(11)],
                8: [lambda: emit_v(12)],
                9: [lambda: emit_v(13)],
                10: [lambda: emit_proj_half(wkT_sb, kT_c, 0, 3),
                     lambda: emit_proj_half(wkT_sb, kT_c, 1, 3)],
                11: [lambda: emit_v(14)],
                12: [lambda: emit_v(15)],
            })
            emit_attn_pair(1, 3, fillers={
                1: [lambda: emit_proj_half(wqT_sb, qT_c, 0, 2)],
                6: [lambda: emit_proj_half(wqT_sb, qT_c, 1, 2)],
            })
            enqueue_out(3)
            emit_attn_pair(0, 2, fillers={
                1: [lambda: emit_proj_half(wqT_sb, qT_c, 0, 1)],
                6: [lambda: emit_proj_half(wqT_sb, qT_c, 1, 1)],
            })
            emit_attn_pair(1, 2)
            enqueue_out(2)
            emit_attn_pair(0, 1, fillers={
                1: [lambda: emit_proj_half(wqT_sb, qT_c, 0, 0)],
                4: [lambda: emit_proj_half(wqT_sb, qT_c, 1, 0)],
            })
            emit_attn_pair(1, 1)
            enqueue_out(1)
            lazy = lambda: (filler_q.pop(0)() if filler_q else None)
            emit_attn_pair(0, 0, fillers={1: [lazy], 2: [lazy]})
            emit_attn_pair(1, 0, fillers={1: [lazy], 2: [lazy]})
            enqueue_out(0)
            while filler_q:
                filler_q.pop(0)()

    nc.compile()
    return nc


def get_nc():
    global _NC_CACHE
    if _NC_CACHE is None:
        _NC_CACHE = _build_nc()
    return _NC_CACHE


def _round_fp32r(a):
    """Round-to-nearest-even onto the fp32r grid (11 mantissa bits)."""
    b = np.ascontiguousarray(a, dtype=np.float32).view(np.uint32)
    b = b + 0x7FF + ((b >> 12) & 1)
    b &= np.uint32(0xFFFFF000)
    return b.view(np.float32)


def _bf16(a):
    return np.ascontiguousarray(a, dtype=np.float32).astype(ml_dtypes.bfloat16)


def make_in_maps(x, Wq, Wk, Wv, Wo):
    x = np.asarray(x, dtype=np.float32)
    Wq = np.asarray(Wq, dtype=np.float32)
    Wk = np.asarray(Wk, dtype=np.float32)
    Wv = np.asarray(Wv, dtype=np.float32)
    Wo = np.asarray(Wo, dtype=np.float32)
    in_maps = []
    for c in range(N_CORES):
        b, g = divmod(c, TP)
        sl = slice(DL * g, DL * (g + 1))
        in_maps.append(
            {
                "xT": _bf16(x[b].T),
                "wqT": _bf16(Wq[sl].T),
                "wkT": _bf16(Wk[sl].T),
                "wvT": _bf16(Wv[sl].T),
                "woT": _round_fp32r(Wo[:, sl].T),
            }
        )
    return in_maps


def _combine(results, bo):
    bo = np.asarray(bo, dtype=np.float32)
    y = np.zeros((B, S, E), dtype=np.float32)
    for c in range(N_CORES):
        y[c // TP] += results[c]["out"].astype(np.float32)
    y += bo
    return y


def kernel(x, Wq, Wk, Wv, Wo, bo):
    nc = get_nc()
    in_maps = make_in_maps(x, Wq, Wk, Wv, Wo)
    res = run_bass_kernel_spmd(nc, in_maps, list(range(N_CORES)))
    return _combine(res.results, bo)


def kernel_traced(x, Wq, Wk, Wv, Wo, bo, trace_cores=None):
    """Like kernel() but with NTFF tracing; returns (output, BassKernelResults)."""
    nc = get_nc()
    in_maps = make_in_maps(x, Wq, Wk, Wv, Wo)
    res = run_bass_kernel_spmd(
        nc, in_maps, list(range(N_CORES)), trace=True, trace_cores=trace_cores
    )
    return _combine(res.results, bo), res


# revision 17
# speedup vs baseline: 1.0741x; 1.0135x over previous
"""Trainium2 Bass kernel for nn_MultiHeadAttention_55894704390646.

Multi-head causal attention, B=2, S=2048, E=1024, H=16 heads, D=64.
Sharding: data-parallel over batch (2 groups) x tensor-parallel over heads
(4 heads per core). Each core computes a partial output-projection result
(row-split Wo); the host sums the 4 partials per batch and adds the bias.

Final design (per core), ~170us HW exec (baseline 215us):
  - x/Wq/Wk/Wv and attention operands bf16 (PSUM fp32); ctx/Wo f32r; out
    partials fp16.
  - reversed chunk order (J=3..0) -> smallest attention chunk last.
  - causal column restriction + gpsimd affine_select for the diagonal
    triangle (no mask tensors, nothing on DVE).
  - ACT runs the exp stream (the attention pace-setter) plus the second
    den-row stage and late out evacuations; DVE does the other PSUM
    evacuations, reciprocals and normalize multiplies.
  - AVs lag scores by 2 at pair starts so the previous pair's normalize
    chain (den->recip->broadcast->mult) overlaps PE work; projection and
    lagged output-projection groups fill remaining PE slack.
  - inputs stream over both hardware DMA queues (sync + scalar), first
    x chunk split across them; out written one DMA per token block.
"""

import sys

if "/opt/trn_rl_repo" not in sys.path:
    sys.path.insert(0, "/opt/trn_rl_repo")

import numpy as np
import ml_dtypes

import concourse.bass as bass
from concourse import bacc
import concourse.mybir as mybir
import concourse.tile as tile
from concourse.bass_utils import run_bass_kernel_spmd

B, S, E, H, D = 2, 2048, 1024, 16, 64
N_CORES = 8
DP = 2                 # batch groups
TP = 4                 # cores per batch group
HL = H // TP           # local heads per core = 4
DL = HL * D            # local head dims = 256
P = 128
NTB = S // P           # token blocks = 16
QC = 512               # query chunk
NQC = S // QC          # query chunks = 4
NKB = QC // P          # k-blocks per q chunk = 4
NPAIR = HL // 2        # head pairs = 2
NEO = E // QC          # output feature chunks of 512 = 2
NKO = E // P           # contraction blocks over E = 8

f32 = mybir.dt.float32
f32r = mybir.dt.float32r
f16 = mybir.dt.float16
bf16 = mybir.dt.bfloat16
EXP = mybir.ActivationFunctionType.Exp
MULT = mybir.AluOpType.mult

_NC_CACHE = None


def _build_nc():
    nc = bacc.Bacc("TRN2", target_bir_lowering=False, debug=False)

    xT = nc.dram_tensor("xT", (E, S), bf16, kind="ExternalInput")
    wqT = nc.dram_tensor("wqT", (E, DL), bf16, kind="ExternalInput")
    wkT = nc.dram_tensor("wkT", (E, DL), bf16, kind="ExternalInput")
    wvT = nc.dram_tensor("wvT", (E, DL), bf16, kind="ExternalInput")
    woT = nc.dram_tensor("woT", (DL, E), f32r, kind="ExternalInput")
    out = nc.dram_tensor("out", (S, E), f16, kind="ExternalOutput")

    with tile.TileContext(nc) as tc:
        with (
            nc.allow_low_precision(reason="bf16/f32r matmuls, fp16 partials"),
            tc.tile_pool(name="big", bufs=1) as big,
            tc.tile_pool(name="work", bufs=6) as work,
            tc.tile_pool(name="work2", bufs=2) as work2,
            tc.tile_pool(name="osb", bufs=3) as osb,
            tc.tile_pool(name="ps_mm", bufs=2, space="PSUM") as ps_mm,
            tc.tile_pool(name="ps_s", bufs=2, space="PSUM") as ps_s,
            tc.tile_pool(name="ps_ctx", bufs=2, space="PSUM") as ps_ctx,
        ):
            # ---- input DMAs on the two hardware DGE queues (sync+scalar),
            # ordered by first consumer; the first x chunk is split across
            # both queues so kproj(0) can start earliest.
            xT_r = xT[:].rearrange("(ko p) (c s) -> p ko c s", p=P, c=NQC)
            wkT_sb = big.tile([P, NKO, DL], bf16, tag="wkT")
            nc.sync.dma_start(wkT_sb[:], wkT[:].rearrange("(ko p) d -> p ko d", p=P))
            xc = [
                big.tile([P, NKO, QC], bf16, tag=f"xc{c}", name=f"xc{c}")
                for c in range(NQC)
            ]
            nc.scalar.dma_start(xc[0][:, 0:4, :], xT_r[:, 0:4, 0, :])
            nc.sync.dma_start(xc[0][:, 4:8, :], xT_r[:, 4:8, 0, :])
            wqT_sb = big.tile([P, NKO, DL], bf16, tag="wqT")
            nc.scalar.dma_start(wqT_sb[:], wqT[:].rearrange("(ko p) d -> p ko d", p=P))
            nc.sync.dma_start(xc[3][:], xT_r[:, :, 3, :])
            wvT_sb = big.tile([P, NKO, DL], bf16, tag="wvT")
            nc.scalar.dma_start(wvT_sb[:], wvT[:].rearrange("(ko p) d -> p ko d", p=P))
            nc.sync.dma_start(xc[1][:], xT_r[:, :, 1, :])
            nc.scalar.dma_start(xc[2][:], xT_r[:, :, 2, :])
            woT_sb = big.tile([P, NPAIR, E], f32r, tag="woT")
            nc.sync.dma_start(woT_sb[:], woT[:].rearrange("(pr p) e -> p pr e", p=P))

            # persistent activation buffers
            qT_c = [[None] * NQC for _ in range(NPAIR)]
            kT_c = [[None] * NQC for _ in range(NPAIR)]
            for pr in range(NPAIR):
                for ch in range(NQC):
                    qT_c[pr][ch] = big.tile(
                        [P, QC], bf16, tag=f"qT{pr}{ch}", name=f"qT{pr}{ch}"
                    )
                    kT_c[pr][ch] = big.tile(
                        [P, QC], bf16, tag=f"kT{pr}{ch}", name=f"kT{pr}{ch}"
                    )
            v_tb = []
            for tb in range(NTB):
                vt = big.tile([P, HL, D + 1], bf16, tag=f"v{tb}", name=f"v{tb}")
                nc.gpsimd.memset(vt[:, :, D], 1.0)
                v_tb.append(vt)
            ctx_J = []
            for J in range(NQC):
                ctx_J.append(
                    big.tile([P, NPAIR, QC], f32r, tag=f"ctxT{J}", name=f"ctxT{J}")
                )

            def emit_proj_half(wt_sb, dst, pr, ch):
                """One pair's q/k projection for one chunk: 8 MMs + evac."""
                pp = ps_mm.tile([P, QC], f32, tag="mm", name=f"pp_{pr}_{ch}")
                for ko in range(NKO):
                    nc.tensor.matmul(
                        pp[:],
                        wt_sb[:, ko, pr * P : (pr + 1) * P],
                        xc[ch][:, ko, :],
                        start=(ko == 0),
                        stop=(ko == NKO - 1),
                    )
                nc.vector.tensor_copy(dst[pr][ch][:], pp[:])

            def emit_v(tb):
                pv_full = ps_mm.tile([P, QC], f32, tag="mm", name=f"pv{tb}")
                pv = pv_full[:, 0:DL]
                for ko in range(NKO):
                    nc.tensor.matmul(
                        pv[:],
                        xc[tb // NKB][:, ko, (tb % NKB) * P : (tb % NKB + 1) * P],
                        wvT_sb[:, ko, :],
                        start=(ko == 0),
                        stop=(ko == NKO - 1),
                    )
                nc.vector.tensor_copy(
                    v_tb[tb][:, :, 0:D],
                    pv[:].rearrange("p (h d) -> p h d", h=HL),
                )

            def scores_group(pr, J, I):
                """Scores + exp (+ causal triangle zero) for k-block I.
                Diagonal blocks only compute query columns >= 128*di."""
                di = I - NKB * J
                q0 = P * di if di >= 0 else 0
                kch = I // NKB
                ik = slice((I % NKB) * P, (I % NKB + 1) * P)
                s = ps_s.tile([P, 2, QC], f32, tag="s", name="s")
                nc.tensor.matmul(
                    s[:, 0, q0:QC],
                    kT_c[pr][kch][0:64, ik],
                    qT_c[pr][J][0:64, q0:QC],
                    start=True,
                    stop=True,
                )
                nc.tensor.matmul(
                    s[:, 1, q0:QC],
                    kT_c[pr][kch][64:128, ik],
                    qT_c[pr][J][64:128, q0:QC],
                    start=True,
                    stop=True,
                )
                pT = work.tile([P, 2, QC], bf16, tag="pT", name="pT")
                nc.scalar.activation(pT[:, :, q0:QC], s[:, :, q0:QC], EXP, scale=0.125)
                if di >= 0:
                    nc.gpsimd.affine_select(
                        out=pT[:, :, q0 : q0 + P],
                        in_=pT[:, :, q0 : q0 + P],
                        compare_op=mybir.AluOpType.is_ge,
                        fill=0.0,
                        base=0,
                        pattern=[[0, 2], [1, P]],
                        channel_multiplier=-1,
                    )
                return (pT, q0)

            def emit_out_block(J, tb):
                """Output projection for one 128-token block: both feature
                halves accumulated, one evac each, ONE dma."""
                o_sb = osb.tile([P, E], f16, tag="o_sb")
                tsl = slice((tb % NKB) * P, (tb % NKB + 1) * P)
                for ec in range(NEO):
                    o_ps = ps_mm.tile([P, QC], f32, tag="mm", name=f"o{J}_{tb}_{ec}")
                    for pr in range(NPAIR):
                        nc.tensor.matmul(
                            o_ps[:],
                            ctx_J[J][:, pr, tsl],
                            woT_sb[:, pr, ec * QC : (ec + 1) * QC],
                            start=(pr == 0),
                            stop=(pr == NPAIR - 1),
                        )
                    if J <= 1:
                        nc.scalar.copy(o_sb[:, ec * QC : (ec + 1) * QC], o_ps[:])
                    else:
                        nc.vector.tensor_copy(
                            o_sb[:, ec * QC : (ec + 1) * QC], o_ps[:]
                        )
                nc.sync.dma_start(out[tb * P : (tb + 1) * P, :], o_sb[:])

            filler_q = []

            def enqueue_out(J):
                for tb in range(NKB * J, NKB * (J + 1)):
                    filler_q.append(lambda J=J, tb=tb: emit_out_block(J, tb))

            def emit_attn_pair(pr, J, fillers=None, trailing=None):
                """Attention for head pair (2pr, 2pr+1) on query chunk J.
                AVs lag scores by 2 for the first iterations (boundary
                decoupling from the previous pair's normalize chain), then
                by 1. Fillers keep the PE fed at the ACT exp pace."""
                h0, h1 = 2 * pr, 2 * pr + 1
                nI = NKB * (J + 1)
                ctx = [
                    ps_ctx.tile([D + 1, QC], f32, tag="ctx", name="ctx0"),
                    ps_ctx.tile([D + 1, QC], f32, tag="ctx", name="ctx1"),
                ]

                def emit_av(I, pTq):
                    pT, q0 = pTq
                    nc.tensor.matmul(
                        ctx[0][:, q0:QC], v_tb[I][:, h0, :], pT[:, 0, q0:QC],
                        start=(I == 0), stop=(I == nI - 1),
                        skip_group_check=True,
                    )
                    nc.tensor.matmul(
                        ctx[1][:, q0:QC], v_tb[I][:, h1, :], pT[:, 1, q0:QC],
                        start=(I == 0), stop=(I == nI - 1),
                        skip_group_check=True,
                    )

                def filler(I):
                    if fillers and I in fillers:
                        for th in fillers[I]:
                            th()
                    elif filler_q and I % 2 == 0 and 4 <= I:
                        filler_q.pop(0)()

                pTs = [pending.pop() if pending else scores_group(pr, J, 0)]
                nxt_av = 0
                for I in range(1, nI):
                    pTs.append(scores_group(pr, J, I))
                    filler(I)
                    if I >= 2:
                        emit_av(nxt_av, pTs.pop(0))
                        nxt_av += 1
                nxt = chain.pop(0) if chain else None
                if nxt is not None:
                    pending.append(scores_group(nxt[0], nxt[1], 0))
                while pTs:
                    emit_av(nxt_av, pTs.pop(0))
                    nxt_av += 1
                if trailing:
                    for th in trailing:
                        th()

                # normalize as two independent per-head chains so each ctx
                # bank frees as early as possible. den staged DVE(h0)/ACT(h1)
                # in parallel; per-head recip + broadcast + multiply.
                ctx0, ctx1 = ctx

                def norm_head(r, cx, first):
                    den_sb = work2.tile([1, QC], f32, tag=f"den{r}", name=f"den{r}")
                    eng = nc.vector if r == 0 else nc.scalar
                    (eng.tensor_copy if r == 0 else eng.copy)(
                        den_sb[:], cx[D : D + 1, :]
                    )
                    rec = work2.tile([1, QC], f32, tag=f"rec{r}", name=f"rec{r}")
                    nc.vector.reciprocal_approx_fast(rec[:], den_sb[:])
                    dnb = work2.tile([64, QC], f32, tag=f"dnb{r}", name=f"dnb{r}")
                    nc.gpsimd.partition_broadcast(dnb[:], rec[:])
                    if r == 0:
                        nc.vector.tensor_tensor(
                            ctx_J[J][0:64, pr, :], cx[0:D, :], dnb[:], MULT
                        )
                    else:
                        tmp = work2.tile([64, QC], f32r, tag="ctmp")
                        nc.vector.tensor_tensor(tmp[:], cx[0:D, :], dnb[:], MULT)
                        nc.sync.dma_start(ctx_J[J][64:128, pr, :], tmp[:])

                if J == 0:
                    norm_head(1, ctx1, True)
                    norm_head(0, ctx0, False)
                else:
                    norm_head(0, ctx0, True)
                    norm_head(1, ctx1, False)

            # ---- schedule: reversed chunk order, fillers keep PE dense
            chain = [(0, 3), (1, 3), (0, 2), (1, 2), (0, 1), (1, 1), (0, 0), (1, 0)]
            pending = []

            emit_proj_half(wkT_sb, kT_c, 0, 0)
            emit_proj_half(wkT_sb, kT_c, 1, 0)
            emit_proj_half(wqT_sb, qT_c, 0, 3)
            emit_proj_half(wqT_sb, qT_c, 1, 3)
            for tb in range(4):
                emit_v(tb)
            chain.pop(0)
            emit_attn_pair(0, 3, fillers={
                1: [lambda: emit_v(4), lambda: emit_v(5)],
                2: [lambda: emit_proj_half(wkT_sb, kT_c, 0, 1),
                    lambda: emit_proj_half(wkT_sb, kT_c, 1, 1)],
                3: [lambda: emit_v(6), lambda: emit_v(7)],
                4: [lambda: emit_v(8)],
                5: [lambda: emit_v(9)],
                6: [lambda: emit_proj_half(wkT_sb, kT_c, 0, 2),
                    lambda: emit_proj_half(wkT_sb, kT_c, 1, 2)],
                7: [lambda: emit_v(10), lambda: emit_v(11)],
                8: [lambda: emit_v(12)],
                9: [lambda: emit_v(13)],
                10: [lambda: emit_proj_half(wkT_sb, kT_c, 0, 3),
                     lambda: emit_proj_half(wkT_sb, kT_c, 1, 3)],
                11: [lambda: emit_v(14)],
                12: [lambda: emit_v(15)],
            })
            emit_attn_pair(1, 3, fillers={
                1: [lambda: emit_proj_half(wqT_sb, qT_c, 0, 2)],
                6: [lambda: emit_proj_half(wqT_sb, qT_c, 1, 2)],
            })
            enqueue_out(3)
            emit_attn_pair(0, 2, fillers={
                1: [lambda: emit_proj_half(wqT_sb, qT_c, 0, 1)],
                6: [lambda: emit_proj_half(wqT_sb, qT_c, 1, 1)],
            })
            emit_attn_pair(1, 2)
            enqueue_out(2)
            emit_attn_pair(0, 1, fillers={
                1: [lambda: emit_proj_half(wqT_sb, qT_c, 0, 0)],
                4: [lambda: emit_proj_half(wqT_sb, qT_c, 1, 0)],
            })
            emit_attn_pair(1, 1)
            enqueue_out(1)
            lazy = lambda: (filler_q.pop(0)() if filler_q else None)
            emit_attn_pair(0, 0, fillers={1: [lazy]})
            emit_attn_pair(1, 0)
            enqueue_out(0)
            while filler_q:
                filler_q.pop(0)()

    nc.compile()
    return nc


def get_nc():
    global _NC_CACHE
    if _NC_CACHE is None:
        _NC_CACHE = _build_nc()
    return _NC_CACHE


def _round_fp32r(a):
    """Round-to-nearest-even onto the fp32r grid (11 mantissa bits)."""
    b = np.ascontiguousarray(a, dtype=np.float32).view(np.uint32)
    b = b + 0x7FF + ((b >> 12) & 1)
    b &= np.uint32(0xFFFFF000)
    return b.view(np.float32)


def _bf16(a):
    return np.ascontiguousarray(a, dtype=np.float32).astype(ml_dtypes.bfloat16)


def make_in_maps(x, Wq, Wk, Wv, Wo):
    x = np.asarray(x, dtype=np.float32)
    Wq = np.asarray(Wq, dtype=np.float32)
    Wk = np.asarray(Wk, dtype=np.float32)
    Wv = np.asarray(Wv, dtype=np.float32)
    Wo = np.asarray(Wo, dtype=np.float32)
    in_maps = []
    for c in range(N_CORES):
        b, g = divmod(c, TP)
        sl = slice(DL * g, DL * (g + 1))
        in_maps.append(
            {
                "xT": _bf16(x[b].T),
                "wqT": _bf16(Wq[sl].T),
                "wkT": _bf16(Wk[sl].T),
                "wvT": _bf16(Wv[sl].T),
                "woT": _round_fp32r(Wo[:, sl].T),
            }
        )
    return in_maps


def _combine(results, bo):
    bo = np.asarray(bo, dtype=np.float32)
    y = np.zeros((B, S, E), dtype=np.float32)
    for c in range(N_CORES):
        y[c // TP] += results[c]["out"].astype(np.float32)
    y += bo
    return y


def kernel(x, Wq, Wk, Wv, Wo, bo):
    nc = get_nc()
    in_maps = make_in_maps(x, Wq, Wk, Wv, Wo)
    res = run_bass_kernel_spmd(nc, in_maps, list(range(N_CORES)))
    return _combine(res.results, bo)


def kernel_traced(x, Wq, Wk, Wv, Wo, bo, trace_cores=None):
    """Like kernel() but with NTFF tracing; returns (output, BassKernelResults)."""
    nc = get_nc()
    in_maps = make_in_maps(x, Wq, Wk, Wv, Wo)
    res = run_bass_kernel_spmd(
        nc, in_maps, list(range(N_CORES)), trace=True, trace_cores=trace_cores
    )
    return _combine(res.results, bo), res
